# revision 12
# baseline (speedup 1.0000x reference)
"""Sinkhorn AssignmentLoss kernel for 8 TRN2 NeuronCores.

Math: the reference's stabilized log-space Sinkhorn is equivalent (exactly,
up to fp rounding) to exp-space Sinkhorn on the positive kernel matrix
  K2 = [exp(logits - g), rowsum(exp(logits - g)) * exp(d - g)]   # [N, C+1]
with per-sample scalar g = max(max(logits), d) (scale invariance lets us drop
the softmax row-normalization into u):
  u = mu / (K2 v);  v = nu / (K2^T u);  P = diag(u) K2 diag(v)
With TEMP=1 the iteration converges in <4 iterations (measured 2e-4 rel err
vs the reference's 20 iterations at ITERS=3, fp16 kernel storage).

Per core: 8 samples, data-parallel over batch (no collectives).
Device pipeline per sample:
  DMA logits -> ACT exp(+rowsum accum) -> fp16 KN [n-part, c-free]
  PE transpose -> fp16 KT [c-part, n-free]
  ITERS x { PE matvec Kv (stream KT), DVE divide, PE row->col tiny matmuls,
            PE matvec K^T u (stream KN), DVE divide, tiny matmuls }
  DVE scalar_tensor_tensor: P = KN * u[n] * v[c] -> DMA out
"""

import sys
import numpy as np

for _p in ("/opt/trn_rl_repo", "/root/.axon_site/_ro/trn_rl_repo"):
    if _p not in sys.path:
        sys.path.insert(0, _p)

from contextlib import ExitStack

import concourse.bass as bass
import concourse.tile as tile
from concourse import bacc, mybir
from concourse.bass_utils import run_bass_kernel_spmd

B, N, C = 64, 1024, 558
CP1 = C + 1
NCORES = 8
S = B // NCORES          # samples per core
NT = N // 128            # 8 row tiles
CW = [128, 128, 128, 128, CP1 - 512]   # c-chunk widths (..., 47)
ITERS = 3
MU_SCALE = 256.0         # keeps u, v in fp16 normal range; cancels exactly in P

F32 = mybir.dt.float32
F16 = mybir.dt.float16
EXP = mybir.ActivationFunctionType.Exp
MULT = mybir.AluOpType.mult
DIV = mybir.AluOpType.divide


def _build_kernel(ctx: ExitStack, tc: "tile.TileContext", out, lg, mu, gneg, edg, ident):
    nc = tc.nc

    singles = ctx.enter_context(tc.tile_pool(name="singles", bufs=1))
    lgp = ctx.enter_context(tc.tile_pool(name="lgp", bufs=2))
    knp = ctx.enter_context(tc.tile_pool(name="knp", bufs=3))
    ktp = ctx.enter_context(tc.tile_pool(name="ktp", bufs=3))
    vecp = ctx.enter_context(tc.tile_pool(name="vecp", bufs=3))
    outp = ctx.enter_context(tc.tile_pool(name="outp", bufs=3))
    ptp = ctx.enter_context(tc.tile_pool(name="ptp", bufs=2, space="PSUM"))
    rowp = ctx.enter_context(tc.tile_pool(name="rowp", bufs=4, space="PSUM"))
    pcp = ctx.enter_context(tc.tile_pool(name="pcp", bufs=2, space="PSUM"))

    sb_ident = singles.tile([128, 128], F16)
    nc.sync.dma_start(sb_ident[:], ident)
    sb_gneg = singles.tile([128, S], F32)
    nc.sync.dma_start(sb_gneg[:], gneg)
    sb_edg = singles.tile([128, S], F32)
    nc.sync.dma_start(sb_edg[:], edg)
    sb_mu = singles.tile([1, S, N], F32)
    nc.sync.dma_start(sb_mu[:], mu)
    # mu and nu both carry MU_SCALE (fixed point: u' = SC*u, v' = v);
    # the final P pass divides v back out via the ones/SC broadcast weights.
    sb_one = singles.tile([1, 1], F16)
    nc.vector.memset(sb_one[:], 1.0)
    sb_ones128 = singles.tile([1, 128], F16)
    nc.vector.memset(sb_ones128[:], 1.0 / MU_SCALE)

    for s in range(S):
        # ---- load logits [1024, 558] as [128, 8, 558] ----
        lgt = lgp.tile([128, NT, C], F32, tag="lgt")
        nc.sync.dma_start(lgt[:], lg[s].rearrange("(t p) c -> p t c", p=128))

        # ---- KN = exp(logits - g) with per-row sums; dustbin col ----
        kn = knp.tile([128, NT, CP1], F16, tag="kn")
        sacc = vecp.tile([128, NT], F32, tag="sacc")
        for t in range(NT):
            nc.scalar.activation(
                kn[:, t, 0:C], lgt[:, t, :], EXP,
                bias=sb_gneg[:, s : s + 1], scale=1.0,
                accum_out=sacc[:, t : t + 1],
            )
        # kn[:, t, C] = sacc[:, t] * exp(d - g)
        nc.vector.tensor_scalar(
            kn[:, :, C], sacc[:], sb_edg[:, s : s + 1], None, MULT
        )

        # ---- KT = KN^T via PE transpose ----
        kt = ktp.tile([128, 5, N], F16, tag="kt")
        for j in range(5):
            w = CW[j]
            pt = ptp.tile([128, N], F16, tag="pt")
            for t in range(NT):
                nc.tensor.transpose(
                    pt[0:w, 128 * t : 128 * (t + 1)],
                    kn[:, t, 128 * j : 128 * j + w],
                    sb_ident[:],
                )
            nc.scalar.copy(kt[0:w, j, :], pt[0:w, :])

        # ---- Sinkhorn iterations ----
        vcol = vecp.tile([128, 5], F16, tag="vcol")
        nc.vector.memset(vcol[:], 1.0)
        pc = None
        vrow = None
        for it in range(ITERS):
            # r = K2 v : stream KT, weights = vcol chunks -> row [1, 1024]
            pu0 = rowp.tile([1, 512], F32, tag="row")
            pu1 = rowp.tile([1, 512], F32, tag="row")
            for j in range(5):
                w = CW[j]
                for h, pu in enumerate((pu0, pu1)):
                    nc.tensor.matmul(
                        pu[:],
                        lhsT=vcol[0:w, j : j + 1],
                        rhs=kt[0:w, j, 512 * h : 512 * (h + 1)],
                        start=(j == 0), stop=(j == 4),
                    )
            # u = mu * (1/r)  (fp16 row); divide is unimplemented on TRN2 DVE
            wu = vecp.tile([1, N], F32, tag="wu")
            nc.vector.reciprocal_approx_fast(wu[:, 0:512], pu0[:])
            nc.vector.reciprocal_approx_fast(wu[:, 512:N], pu1[:])
            urow = vecp.tile([1, N], F16, tag="urow")
            nc.vector.tensor_mul(urow[:, 0:512], sb_mu[0:1, s, 0:512], wu[:, 0:512])
            nc.vector.tensor_mul(urow[:, 512:N], sb_mu[0:1, s, 512:N], wu[:, 512:N])
            # u row -> columns [128, NT] via rank-1 matmuls
            pc = pcp.tile([128, NT + 5], F32, tag="pc")
            for t in range(NT):
                nc.tensor.matmul(
                    pc[:, t : t + 1],
                    lhsT=urow[0:1, 128 * t : 128 * (t + 1)],
                    rhs=sb_one[:],
                    start=True, stop=True,
                )
            ucol = vecp.tile([128, NT], F16, tag="ucol")
            nc.vector.tensor_copy(ucol[:], pc[:, 0:NT])
            # sc = K2^T u : stream KN, weights = ucol chunks -> row [1, 559]
            pv0 = rowp.tile([1, 512], F32, tag="row")
            pv1 = rowp.tile([1, 512], F32, tag="row")
            for t in range(NT):
                nc.tensor.matmul(
                    pv0[:], lhsT=ucol[:, t : t + 1], rhs=kn[:, t, 0:512],
                    start=(t == 0), stop=(t == NT - 1),
                )
                nc.tensor.matmul(
                    pv1[0:1, 0 : CP1 - 512], lhsT=ucol[:, t : t + 1],
                    rhs=kn[:, t, 512:CP1],
                    start=(t == 0), stop=(t == NT - 1),
                )
            # v = nu * (1/sc)  (fp16 row)
            wv = vecp.tile([1, CP1], F32, tag="wv")
            nc.vector.reciprocal_approx_fast(wv[:, 0:512], pv0[:])
            nc.vector.reciprocal_approx_fast(wv[:, 512:CP1], pv1[0:1, 0 : CP1 - 512])
            vrow = vecp.tile([1, CP1], F16, tag="vrow")
            nc.vector.tensor_scalar_mul(vrow[:], wv[:], MU_SCALE / CP1)
            # v row -> columns [128, 5]
            for j in range(5):
                w = CW[j]
                nc.tensor.matmul(
                    pc[0:w, NT + j : NT + j + 1],
                    lhsT=vrow[0:1, 128 * j : 128 * j + w],
                    rhs=sb_one[:],
                    start=True, stop=True,
                )
            vcol = vecp.tile([128, 5], F16, tag="vcol")
            nc.vector.tensor_copy(vcol[:, 0:4], pc[:, NT : NT + 4])
            nc.vector.tensor_copy(
                vcol[0 : CW[4], 4:5], pc[0 : CW[4], NT + 4 : NT + 5]
            )

        # ---- P = KN * u[n] * v[c]; DMA out ----
        # broadcast v across partitions: rank-1 matmul ones[128] x vrow
        pvrep0 = rowp.tile([128, 512], F32, tag="row")
        pvrep1 = rowp.tile([128, CP1 - 512], F32, tag="row")
        nc.tensor.matmul(
            pvrep0[:], lhsT=sb_ones128[:], rhs=vrow[0:1, 0:512],
            start=True, stop=True,
        )
        nc.tensor.matmul(
            pvrep1[:], lhsT=sb_ones128[:], rhs=vrow[0:1, 512:CP1],
            start=True, stop=True,
        )
        for t in range(NT):
            po = outp.tile([128, CP1], F32, tag="po")
            nc.vector.scalar_tensor_tensor(
                po[:, 0:512], kn[:, t, 0:512], pc[:, t : t + 1], pvrep0[:], MULT, MULT
            )
            nc.vector.scalar_tensor_tensor(
                po[:, 512:CP1], kn[:, t, 512:CP1], pc[:, t : t + 1], pvrep1[:],
                MULT, MULT,
            )
            nc.sync.dma_start(out[s, 128 * t : 128 * (t + 1), :], po[:])


_NC_CACHE = None


def _get_nc():
    global _NC_CACHE
    if _NC_CACHE is not None:
        return _NC_CACHE
    nc = bacc.Bacc(
        "TRN2", target_bir_lowering=False, debug=False,
        enable_asserts=False, num_devices=NCORES,
    )
    lg = nc.dram_tensor("logits", [S, N, C], F32, kind="ExternalInput").ap()
    mu = nc.dram_tensor("mu", [1, S, N], F32, kind="ExternalInput").ap()
    gneg = nc.dram_tensor("gneg", [128, S], F32, kind="ExternalInput").ap()
    edg = nc.dram_tensor("edg", [128, S], F32, kind="ExternalInput").ap()
    ident = nc.dram_tensor("ident", [128, 128], F16, kind="ExternalInput").ap()
    out = nc.dram_tensor("out", [S, N, CP1], F32, kind="ExternalOutput").ap()
    with tile.TileContext(nc) as tc, ExitStack() as ctx:
        _build_kernel(ctx, tc, out, lg, mu, gneg, edg, ident)
    nc.compile()
    _NC_CACHE = nc
    return nc


def make_in_maps(logits, visible_mask, dustbin_col_score):
    logits = np.ascontiguousarray(np.asarray(logits, dtype=np.float32))
    mask = np.asarray(visible_mask).astype(bool)
    d = float(np.asarray(dustbin_col_score).reshape(-1)[0])
    g = np.maximum(logits.max(axis=(1, 2)), d).astype(np.float32)      # [B]
    nv = mask.sum(-1).astype(np.float32)
    mu = (MU_SCALE * mask / np.maximum(nv, 1.0)[:, None]).astype(np.float32)
    gneg = np.repeat(-g[None, :], 128, axis=0).astype(np.float32)      # [128, B]
    edg = np.repeat(np.exp(d - g)[None, :], 128, axis=0).astype(np.float32)
    ident = np.eye(128, dtype=np.float16)
    in_maps = []
    for i in range(NCORES):
        sl = slice(i * S, (i + 1) * S)
        in_maps.append({
            "logits": logits[sl],
            "mu": np.ascontiguousarray(mu[sl][None]),
            "gneg": np.ascontiguousarray(gneg[:, sl]),
            "edg": np.ascontiguousarray(edg[:, sl]),
            "ident": ident,
        })
    return in_maps


def kernel(logits, visible_mask, dustbin_col_score):
    nc = _get_nc()
    in_maps = make_in_maps(logits, visible_mask, dustbin_col_score)
    res = run_bass_kernel_spmd(nc, in_maps, core_ids=list(range(NCORES)))
    P = np.concatenate([res.results[i]["out"] for i in range(NCORES)], axis=0)
    return np.ascontiguousarray(P.astype(np.float32))


# revision 16
# speedup vs baseline: 1.1487x; 1.1487x over previous
"""Sinkhorn AssignmentLoss kernel for 8 TRN2 NeuronCores.

Math: the reference's stabilized log-space Sinkhorn is equivalent (exactly,
up to fp rounding) to exp-space Sinkhorn on the positive kernel matrix
  K2 = [exp(logits - g), rowsum(exp(logits - g)) * exp(d - g)]   # [N, C+1]
with per-sample scalar g = max(max(logits), d) (scale invariance lets us drop
the softmax row-normalization into u):
  u = mu / (K2 v);  v = nu / (K2^T u);  P = diag(u) K2 diag(v)
With TEMP=1 the iteration converges in <4 iterations (measured ~4e-4 rel err
vs the reference's 20 iterations at ITERS=3, fp16 kernel storage).

Per core: 8 samples, data-parallel over batch (no collectives).
Device pipeline per sample:
  DMA logits -> ACT exp(+rowsum accum) -> fp16 KN [n-part, c-free]
  PE transpose -> fp16 KT [c-part, n-free]
  ITERS x weights-form matvecs: K chunks are PE weights (fp16 FWL),
    u/v column vectors [128, k] are the 1-wide moving operand, so matvec
    results land as PSUM columns and the division (reciprocal_approx_fast
    + multiply) runs on all 128 DVE lanes.
  P = KN * u[n] * v[c] via scalar_tensor_tensor -> DMA out
"""

import sys
import numpy as np

for _p in ("/opt/trn_rl_repo", "/root/.axon_site/_ro/trn_rl_repo"):
    if _p not in sys.path:
        sys.path.insert(0, _p)

from contextlib import ExitStack

import concourse.bass as bass
import concourse.tile as tile
from concourse import bacc, mybir
from concourse.bass_utils import run_bass_kernel_spmd

B, N, C = 64, 1024, 558
CP1 = C + 1
NCORES = 8
S = B // NCORES          # samples per core
NT = N // 128            # 8 row tiles
CW = [128, 128, 128, 128, CP1 - 512]   # c-chunk widths (..., 47)
ITERS = 3
MU_SCALE = 256.0         # keeps u, v in fp16 normal range; cancels exactly in P

F32 = mybir.dt.float32
F16 = mybir.dt.float16
EXP = mybir.ActivationFunctionType.Exp
MULT = mybir.AluOpType.mult


def _build_kernel(ctx: ExitStack, tc: "tile.TileContext", out, lg, mu, gneg, edg, ident):
    nc = tc.nc

    singles = ctx.enter_context(tc.tile_pool(name="singles", bufs=1))
    lgp = ctx.enter_context(tc.tile_pool(name="lgp", bufs=2))
    knp = ctx.enter_context(tc.tile_pool(name="knp", bufs=3))
    ktp = ctx.enter_context(tc.tile_pool(name="ktp", bufs=3))
    vecp = ctx.enter_context(tc.tile_pool(name="vecp", bufs=3))
    outp = ctx.enter_context(tc.tile_pool(name="outp", bufs=3))
    ptp = ctx.enter_context(tc.tile_pool(name="ptp", bufs=2, space="PSUM"))
    pup = ctx.enter_context(tc.tile_pool(name="pup", bufs=2, space="PSUM"))
    pvp = ctx.enter_context(tc.tile_pool(name="pvp", bufs=2, space="PSUM"))
    prp = ctx.enter_context(tc.tile_pool(name="prp", bufs=2, space="PSUM"))

    sb_ident = singles.tile([128, 128], F16)
    nc.sync.dma_start(sb_ident[:], ident)
    sb_gneg = singles.tile([128, S], F32)
    nc.sync.dma_start(sb_gneg[:], gneg)
    sb_edg = singles.tile([128, S], F32)
    nc.sync.dma_start(sb_edg[:], edg)
    # mu in column layout: mucol[p, s, t] = MU_SCALE * mask/nv at row 128*t+p
    sb_mu = singles.tile([128, S, NT], F32)
    nc.sync.dma_start(sb_mu[:], mu)
    # broadcast weights carry 1/MU_SCALE so P = kn * u' * v'/SC
    sb_ones128 = singles.tile([1, 128], F16)
    nc.vector.memset(sb_ones128[:], 1.0 / MU_SCALE)

    for s in range(S):
        # ---- load logits [1024, 558] as [128, 8, 558] ----
        lgt = lgp.tile([128, NT, C], F32, tag="lgt")
        nc.sync.dma_start(lgt[:], lg[s].rearrange("(t p) c -> p t c", p=128))

        # ---- KN = exp(logits - g) with per-row sums; dustbin col ----
        kn = knp.tile([128, NT, CP1], F16, tag="kn")
        sacc = vecp.tile([128, NT], F32, tag="sacc")
        for t in range(NT):
            nc.scalar.activation(
                kn[:, t, 0:C], lgt[:, t, :], EXP,
                bias=sb_gneg[:, s : s + 1], scale=1.0,
                accum_out=sacc[:, t : t + 1],
            )
        # kn[:, t, C] = sacc[:, t] * exp(d - g)
        nc.vector.tensor_scalar(
            kn[:, :, C], sacc[:], sb_edg[:, s : s + 1], None, MULT
        )

        # ---- KT = KN^T via PE transpose; PSUM->SBUF copies split ACT/DVE ----
        kt = ktp.tile([128, 5, N], F16, tag="kt")
        for j in range(5):
            w = CW[j]
            pt = ptp.tile([128, N], F16, tag="pt")
            for t in range(NT):
                nc.tensor.transpose(
                    pt[0:w, 128 * t : 128 * (t + 1)],
                    kn[:, t, 128 * j : 128 * j + w],
                    sb_ident[:],
                )
            if j % 2 == 0:
                nc.scalar.copy(kt[0:w, j, :], pt[0:w, :])
            else:
                nc.vector.tensor_copy(kt[0:w, j, :], pt[0:w, :])

        # ---- Sinkhorn iterations (weights-form matvecs) ----
        vcol = vecp.tile([128, 5], F16, tag="vcol")
        nc.vector.memset(vcol[:], 1.0)
        ucol = None
        for it in range(ITERS):
            # r = K2 v : weights = KT chunks, moving = vcol -> columns [128, NT]
            pu = pup.tile([128, NT], F32, tag="pu")
            for t in range(NT):
                for j in range(5):
                    w = CW[j]
                    nc.tensor.matmul(
                        pu[:, t : t + 1],
                        lhsT=kt[0:w, j, 128 * t : 128 * (t + 1)],
                        rhs=vcol[0:w, j : j + 1],
                        start=(j == 0), stop=(j == 4),
                    )
            # u = mu * (1/r), fp16 columns
            wu = vecp.tile([128, NT], F32, tag="wu")
            nc.vector.reciprocal_approx_fast(wu[:], pu[:])
            ucol = vecp.tile([128, NT], F16, tag="ucol")
            nc.vector.tensor_mul(ucol[:], sb_mu[:, s, :], wu[:])
            # sc = K2^T u : weights = KN chunks, moving = ucol -> columns [128, 5]
            pv = pvp.tile([128, 5], F32, tag="pv")
            for j in range(5):
                w = CW[j]
                for t in range(NT):
                    nc.tensor.matmul(
                        pv[0:w, j : j + 1],
                        lhsT=kn[:, t, 128 * j : 128 * j + w],
                        rhs=ucol[:, t : t + 1],
                        start=(t == 0), stop=(t == NT - 1),
                    )
            # v = nu * (1/sc), fp16 columns
            wv = vecp.tile([128, 5], F32, tag="wv")
            nc.vector.reciprocal_approx_fast(wv[:, 0:4], pv[:, 0:4])
            nc.vector.reciprocal_approx_fast(wv[0 : CW[4], 4:5], pv[0 : CW[4], 4:5])
            vcol = vecp.tile([128, 5], F16, tag="vcol")
            nc.vector.tensor_scalar(
                vcol[:, 0:4], wv[:, 0:4], MU_SCALE / CP1, None, MULT
            )
            nc.vector.tensor_scalar(
                vcol[0 : CW[4], 4:5], wv[0 : CW[4], 4:5], MU_SCALE / CP1, None, MULT
            )

        # ---- P = KN * u[n] * v[c]/SC; DMA out ----
        # v columns -> one row on partition 0, then broadcast across partitions
        ptv = ptp.tile([128, N], F16, tag="pt")
        for j in range(5):
            w = CW[j]
            nc.tensor.transpose(
                ptv[0:1, 128 * j : 128 * j + w], vcol[0:w, j : j + 1],
                sb_ident[0:w, 0:w],
            )
        vsb = vecp.tile([1, 640], F16, tag="vsb")
        nc.vector.tensor_copy(vsb[:, 0:CP1], ptv[0:1, 0:CP1])
        pr0 = prp.tile([128, 512], F32, tag="pr")
        pr1 = prp.tile([128, CW[4]], F32, tag="pr")
        for j in range(5):
            w = CW[j]
            dst = pr0[:, 128 * j : 128 * j + w] if j < 4 else pr1[:]
            nc.tensor.matmul(
                dst, lhsT=sb_ones128[:], rhs=vsb[0:1, 128 * j : 128 * j + w],
                start=True, stop=True,
            )
        for t in range(NT):
            po = outp.tile([128, CP1], F32, tag="po")
            nc.vector.scalar_tensor_tensor(
                po[:, 0:512], kn[:, t, 0:512], ucol[:, t : t + 1], pr0[:], MULT, MULT
            )
            nc.vector.scalar_tensor_tensor(
                po[:, 512:CP1], kn[:, t, 512:CP1], ucol[:, t : t + 1], pr1[:],
                MULT, MULT,
            )
            nc.sync.dma_start(out[s, 128 * t : 128 * (t + 1), :], po[:])


_NC_CACHE = None


def _get_nc():
    global _NC_CACHE
    if _NC_CACHE is not None:
        return _NC_CACHE
    nc = bacc.Bacc(
        "TRN2", target_bir_lowering=False, debug=False,
        enable_asserts=False, num_devices=NCORES,
    )
    lg = nc.dram_tensor("logits", [S, N, C], F32, kind="ExternalInput").ap()
    mu = nc.dram_tensor("mu", [128, S, NT], F32, kind="ExternalInput").ap()
    gneg = nc.dram_tensor("gneg", [128, S], F32, kind="ExternalInput").ap()
    edg = nc.dram_tensor("edg", [128, S], F32, kind="ExternalInput").ap()
    ident = nc.dram_tensor("ident", [128, 128], F16, kind="ExternalInput").ap()
    out = nc.dram_tensor("out", [S, N, CP1], F32, kind="ExternalOutput").ap()
    with tile.TileContext(nc) as tc, ExitStack() as ctx:
        _build_kernel(ctx, tc, out, lg, mu, gneg, edg, ident)
    nc.compile()
    _NC_CACHE = nc
    return nc


def make_in_maps(logits, visible_mask, dustbin_col_score):
    logits = np.ascontiguousarray(np.asarray(logits, dtype=np.float32))
    mask = np.asarray(visible_mask).astype(bool)
    d = float(np.asarray(dustbin_col_score).reshape(-1)[0])
    g = np.maximum(logits.max(axis=(1, 2)), d).astype(np.float32)      # [B]
    nv = mask.sum(-1).astype(np.float32)
    mu = (MU_SCALE * mask / np.maximum(nv, 1.0)[:, None]).astype(np.float32)
    # column layout per core: mucol[p, s, t] = mu[core*S+s, 128*t+p]
    mucol = np.ascontiguousarray(
        mu.reshape(B, NT, 128).transpose(2, 0, 1)
    ).astype(np.float32)                                               # [128, B, NT]
    gneg = np.repeat(-g[None, :], 128, axis=0).astype(np.float32)      # [128, B]
    edg = np.repeat(np.exp(d - g)[None, :], 128, axis=0).astype(np.float32)
    ident = np.eye(128, dtype=np.float16)
    in_maps = []
    for i in range(NCORES):
        sl = slice(i * S, (i + 1) * S)
        in_maps.append({
            "logits": logits[sl],
            "mu": np.ascontiguousarray(mucol[:, sl, :]),
            "gneg": np.ascontiguousarray(gneg[:, sl]),
            "edg": np.ascontiguousarray(edg[:, sl]),
            "ident": ident,
        })
    return in_maps


def kernel(logits, visible_mask, dustbin_col_score):
    nc = _get_nc()
    in_maps = make_in_maps(logits, visible_mask, dustbin_col_score)
    res = run_bass_kernel_spmd(nc, in_maps, core_ids=list(range(NCORES)))
    P = np.concatenate([res.results[i]["out"] for i in range(NCORES)], axis=0)
    return np.ascontiguousarray(P.astype(np.float32))


# revision 19
# speedup vs baseline: 1.6147x; 1.4057x over previous
"""Sinkhorn AssignmentLoss kernel for 8 TRN2 NeuronCores.

Math: the reference's stabilized log-space Sinkhorn is equivalent (exactly,
up to fp rounding) to exp-space Sinkhorn on the positive kernel matrix
  K2 = [exp(logits - g), rowsum(exp(logits - g)) * exp(d - g)]   # [N, C+1]
with per-sample scalar g = max(max(logits), d) (scale invariance lets us drop
the softmax row-normalization into u):
  u = mu / (K2 v);  v = nu / (K2^T u);  P = diag(u) K2 diag(v)
With TEMP=1 the iteration converges in <4 iterations (measured ~4e-4 rel err
vs the reference's 20 iterations at ITERS=3, fp16 kernel storage).

Per core: 8 samples, data-parallel over batch (no collectives).
Samples are processed in interleaved pairs so the PE never stalls on the
short DVE division steps between matvec directions.

Device pipeline per sample:
  DMA logits -> ACT exp(+rowsum accum) -> fp16 KN [n-part, c-free],
    padded to 640 cols with zeros so every weight chunk is 128 wide (FWL)
  PE transpose -> fp16 KT [c-part, n-free]
  ITERS x weights-form matvecs: K chunks are PE weights (fp16 FWL),
    u/v column vectors [128, k] are the 1-wide moving operand, so matvec
    results land as PSUM columns and the division (reciprocal_approx_fast
    + multiply) runs on all 128 DVE lanes.
  P = KN * u[n] * v[c] via fp16 scalar_tensor_tensor (2x mode) -> DMA out
  (fp16 output; host upcasts to fp32 — errors stay ~1e-4 of max|P|)
"""

import sys
import numpy as np

for _p in ("/opt/trn_rl_repo", "/root/.axon_site/_ro/trn_rl_repo"):
    if _p not in sys.path:
        sys.path.insert(0, _p)

from contextlib import ExitStack

import concourse.bass as bass
import concourse.tile as tile
from concourse import bacc, mybir
from concourse.bass_utils import run_bass_kernel_spmd

B, N, C = 64, 1024, 558
CP1 = C + 1
CPAD = 640               # KN free size: 5 chunks of 128
NCORES = 8
S = B // NCORES          # samples per core
NT = N // 128            # 8 row tiles
CW = [128, 128, 128, 128, CP1 - 512]   # logical c-chunk widths (..., 47)
ITERS = 3
MU_SCALE = 256.0         # keeps u, v in fp16 normal range; cancels exactly in P

F32 = mybir.dt.float32
F16 = mybir.dt.float16
EXP = mybir.ActivationFunctionType.Exp
MULT = mybir.AluOpType.mult


def _emit_load(nc, pools, s, lg):
    lgt = pools["lgp"].tile([128, NT, C], F32, tag="lgt")
    nc.sync.dma_start(lgt[:], lg[s].rearrange("(t p) c -> p t c", p=128))
    return lgt


def _emit_build_kn(nc, pools, s, lgt, sb_gneg, sb_edg):
    """exp + rowsum + dustbin + zero-pad -> kn [128, NT, CPAD] f16."""
    kn = pools["knp"].tile([128, NT, CPAD], F16, tag="kn")
    sacc = pools["vecp"].tile([128, NT], F32, tag="sacc")
    nc.gpsimd.memset(kn[:, :, CP1:CPAD], 0.0)
    for t in range(NT):
        nc.scalar.activation(
            kn[:, t, 0:C], lgt[:, t, :], EXP,
            bias=sb_gneg[:, s : s + 1], scale=1.0,
            accum_out=sacc[:, t : t + 1],
        )
    nc.vector.tensor_scalar(
        kn[:, :, C], sacc[:], sb_edg[:, s : s + 1], None, MULT
    )
    return kn


def _emit_transpose(nc, pools, kn, sb_ident):
    """KT = KN^T (incl. zero pad rows) -> kt [128, 5, N] f16."""
    kt = pools["ktp"].tile([128, 5, N], F16, tag="kt")
    for j in range(5):
        pt = pools["ptp"].tile([128, N], F16, tag="pt")
        for t in range(NT):
            nc.tensor.transpose(
                pt[:, 128 * t : 128 * (t + 1)],
                kn[:, t, 128 * j : 128 * (j + 1)],
                sb_ident[:],
            )
        if j % 2 == 0:
            nc.scalar.copy(kt[:, j, :], pt[:])
        else:
            nc.vector.tensor_copy(kt[:, j, :], pt[:])
    return kt


def _emit_vcol_init(nc, pools):
    vcol = pools["vecp"].tile([128, 5], F16, tag="vcol")
    nc.vector.memset(vcol[:], 1.0)
    nc.vector.memset(vcol[:, 4:5], 0.0)
    nc.vector.memset(vcol[0 : CW[4], 4:5], 1.0)
    return vcol


def _emit_kv(nc, pools, kt, vcol):
    """pu[:, t] = sum_j KT[:, j, t-slice]^T v_j  (weights-form)."""
    pu = pools["pup"].tile([128, NT], F32, tag="pu")
    for t in range(NT):
        for j in range(5):
            nc.tensor.matmul(
                pu[:, t : t + 1],
                lhsT=kt[:, j, 128 * t : 128 * (t + 1)],
                rhs=vcol[:, j : j + 1],
                start=(j == 0), stop=(j == 4),
            )
    return pu


def _emit_u(nc, pools, s, pu, sb_mu):
    wu = pools["vecp"].tile([128, NT], F32, tag="wu")
    nc.vector.reciprocal_approx_fast(wu[:], pu[:])
    ucol = pools["vecp"].tile([128, NT], F16, tag="ucol")
    nc.vector.tensor_mul(ucol[:], sb_mu[:, s, :], wu[:])
    return ucol


def _emit_ktu(nc, pools, kn, ucol):
    pv = pools["pvp"].tile([128, 5], F32, tag="pv")
    for j in range(5):
        for t in range(NT):
            nc.tensor.matmul(
                pv[:, j : j + 1],
                lhsT=kn[:, t, 128 * j : 128 * (j + 1)],
                rhs=ucol[:, t : t + 1],
                start=(t == 0), stop=(t == NT - 1),
            )
    return pv


def _emit_v(nc, pools, pv):
    w4 = CW[4]
    wv = pools["vecp"].tile([128, 5], F32, tag="wv")
    nc.vector.reciprocal_approx_fast(wv[:, 0:4], pv[:, 0:4])
    nc.vector.reciprocal_approx_fast(wv[0:w4, 4:5], pv[0:w4, 4:5])
    vcol = pools["vecp"].tile([128, 5], F16, tag="vcol")
    # zero col 4 first so dead lanes (rows w4:128) never hold inf/nan for
    # the padded j=4 matvec; the valid rows are then overwritten below
    nc.vector.memset(vcol[:, 4:5], 0.0)
    nc.vector.tensor_scalar(vcol[:, 0:4], wv[:, 0:4], MU_SCALE / CP1, None, MULT)
    nc.vector.tensor_scalar(
        vcol[0:w4, 4:5], wv[0:w4, 4:5], MU_SCALE / CP1, None, MULT
    )
    return vcol


def _emit_p_out(nc, pools, s, kn, ucol, vcol, sb_ident, sb_ones128, out):
    """P = kn * u[n] * v[c]/SC -> fp16 -> DMA."""
    ptv = pools["ptp"].tile([128, N], F16, tag="pt")
    for j in range(5):
        w = CW[j]
        nc.tensor.transpose(
            ptv[0:1, 128 * j : 128 * j + w], vcol[0:w, j : j + 1],
            sb_ident[0:w, 0:w],
        )
    vsb = pools["vecp"].tile([1, 640], F16, tag="vsb")
    nc.vector.tensor_copy(vsb[:, 0:CP1], ptv[0:1, 0:CP1])
    pr0 = pools["prp"].tile([128, 512], F32, tag="pr")
    pr1 = pools["prp"].tile([128, CW[4]], F32, tag="pr")
    for j in range(5):
        w = CW[j]
        dst = pr0[:, 128 * j : 128 * j + w] if j < 4 else pr1[:]
        nc.tensor.matmul(
            dst, lhsT=sb_ones128[:], rhs=vsb[0:1, 128 * j : 128 * j + w],
            start=True, stop=True,
        )
    vrep0 = pools["vecp"].tile([128, 512], F16, tag="vrep0")
    nc.vector.tensor_copy(vrep0[:], pr0[:])
    vrep1 = pools["vecp"].tile([128, CW[4]], F16, tag="vrep1")
    nc.vector.tensor_copy(vrep1[:], pr1[:])
    for t in range(NT):
        po = pools["outp"].tile([128, CP1], F16, tag="po")
        nc.vector.scalar_tensor_tensor(
            po[:, 0:512], kn[:, t, 0:512], ucol[:, t : t + 1], vrep0[:],
            MULT, MULT,
        )
        nc.vector.scalar_tensor_tensor(
            po[:, 512:CP1], kn[:, t, 512:CP1], ucol[:, t : t + 1], vrep1[:],
            MULT, MULT,
        )
        nc.sync.dma_start(out[s, 128 * t : 128 * (t + 1), :], po[:])


def _build_kernel(ctx: ExitStack, tc: "tile.TileContext", out, lg, mu, gneg, edg, ident):
    nc = tc.nc

    pools = {
        "singles": ctx.enter_context(tc.tile_pool(name="singles", bufs=1)),
        "lgp": ctx.enter_context(tc.tile_pool(name="lgp", bufs=3)),
        "knp": ctx.enter_context(tc.tile_pool(name="knp", bufs=4)),
        "ktp": ctx.enter_context(tc.tile_pool(name="ktp", bufs=4)),
        "vecp": ctx.enter_context(tc.tile_pool(name="vecp", bufs=3)),
        "outp": ctx.enter_context(tc.tile_pool(name="outp", bufs=4)),
        "ptp": ctx.enter_context(tc.tile_pool(name="ptp", bufs=2, space="PSUM")),
        "pup": ctx.enter_context(tc.tile_pool(name="pup", bufs=2, space="PSUM")),
        "pvp": ctx.enter_context(tc.tile_pool(name="pvp", bufs=2, space="PSUM")),
        "prp": ctx.enter_context(tc.tile_pool(name="prp", bufs=2, space="PSUM")),
    }
    singles = pools["singles"]

    sb_ident = singles.tile([128, 128], F16)
    nc.sync.dma_start(sb_ident[:], ident)
    sb_gneg = singles.tile([128, S], F32)
    nc.sync.dma_start(sb_gneg[:], gneg)
    sb_edg = singles.tile([128, S], F32)
    nc.sync.dma_start(sb_edg[:], edg)
    # mu in column layout: mucol[p, s, t] = MU_SCALE * mask/nv at row 128*t+p
    sb_mu = singles.tile([128, S, NT], F32)
    nc.sync.dma_start(sb_mu[:], mu)
    # broadcast weights carry 1/MU_SCALE so P = kn * u' * v'/SC
    sb_ones128 = singles.tile([1, 128], F16)
    nc.vector.memset(sb_ones128[:], 1.0 / MU_SCALE)

    for p in range(S // 2):
        sA, sB = 2 * p, 2 * p + 1
        lgA = _emit_load(nc, pools, sA, lg)
        lgB = _emit_load(nc, pools, sB, lg)
        knA = _emit_build_kn(nc, pools, sA, lgA, sb_gneg, sb_edg)
        knB = _emit_build_kn(nc, pools, sB, lgB, sb_gneg, sb_edg)
        ktA = _emit_transpose(nc, pools, knA, sb_ident)
        ktB = _emit_transpose(nc, pools, knB, sb_ident)
        vcA = _emit_vcol_init(nc, pools)
        vcB = _emit_vcol_init(nc, pools)
        uA = uB = None
        for it in range(ITERS):
            puA = _emit_kv(nc, pools, ktA, vcA)
            puB = _emit_kv(nc, pools, ktB, vcB)
            uA = _emit_u(nc, pools, sA, puA, sb_mu)
            pvA = _emit_ktu(nc, pools, knA, uA)
            uB = _emit_u(nc, pools, sB, puB, sb_mu)
            pvB = _emit_ktu(nc, pools, knB, uB)
            vcA = _emit_v(nc, pools, pvA)
            vcB = _emit_v(nc, pools, pvB)
        _emit_p_out(nc, pools, sA, knA, uA, vcA, sb_ident, sb_ones128, out)
        _emit_p_out(nc, pools, sB, knB, uB, vcB, sb_ident, sb_ones128, out)


_NC_CACHE = None


def _get_nc():
    global _NC_CACHE
    if _NC_CACHE is not None:
        return _NC_CACHE
    nc = bacc.Bacc(
        "TRN2", target_bir_lowering=False, debug=False,
        enable_asserts=False, num_devices=NCORES,
    )
    lg = nc.dram_tensor("logits", [S, N, C], F32, kind="ExternalInput").ap()
    mu = nc.dram_tensor("mu", [128, S, NT], F32, kind="ExternalInput").ap()
    gneg = nc.dram_tensor("gneg", [128, S], F32, kind="ExternalInput").ap()
    edg = nc.dram_tensor("edg", [128, S], F32, kind="ExternalInput").ap()
    ident = nc.dram_tensor("ident", [128, 128], F16, kind="ExternalInput").ap()
    out = nc.dram_tensor("out", [S, N, CP1], F16, kind="ExternalOutput").ap()
    with tile.TileContext(nc) as tc, ExitStack() as ctx:
        _build_kernel(ctx, tc, out, lg, mu, gneg, edg, ident)
    nc.compile()
    _NC_CACHE = nc
    return nc


def make_in_maps(logits, visible_mask, dustbin_col_score):
    logits = np.ascontiguousarray(np.asarray(logits, dtype=np.float32))
    mask = np.asarray(visible_mask).astype(bool)
    d = float(np.asarray(dustbin_col_score).reshape(-1)[0])
    g = np.maximum(logits.max(axis=(1, 2)), d).astype(np.float32)      # [B]
    nv = mask.sum(-1).astype(np.float32)
    mu = (MU_SCALE * mask / np.maximum(nv, 1.0)[:, None]).astype(np.float32)
    # column layout per core: mucol[p, s, t] = mu[core*S+s, 128*t+p]
    mucol = np.ascontiguousarray(
        mu.reshape(B, NT, 128).transpose(2, 0, 1)
    ).astype(np.float32)                                               # [128, B, NT]
    gneg = np.repeat(-g[None, :], 128, axis=0).astype(np.float32)      # [128, B]
    edg = np.repeat(np.exp(d - g)[None, :], 128, axis=0).astype(np.float32)
    ident = np.eye(128, dtype=np.float16)
    in_maps = []
    for i in range(NCORES):
        sl = slice(i * S, (i + 1) * S)
        in_maps.append({
            "logits": logits[sl],
            "mu": np.ascontiguousarray(mucol[:, sl, :]),
            "gneg": np.ascontiguousarray(gneg[:, sl]),
            "edg": np.ascontiguousarray(edg[:, sl]),
            "ident": ident,
        })
    return in_maps


def kernel(logits, visible_mask, dustbin_col_score):
    nc = _get_nc()
    in_maps = make_in_maps(logits, visible_mask, dustbin_col_score)
    res = run_bass_kernel_spmd(nc, in_maps, core_ids=list(range(NCORES)))
    P = np.concatenate([res.results[i]["out"] for i in range(NCORES)], axis=0)
    return np.ascontiguousarray(P.astype(np.float32))
